# revision 49
# baseline (speedup 1.0000x reference)
"""Trainium2 Bass kernel for CausalSelfAttention (GQA + RMSNorm + partial RoPE).

Sharding: 8 cores = (batch b in 0..3) x (kv-head group g in 0..1).
Each core computes the full attention for its (b, g) slice and the partial
output projection over its head columns; the host sums the two partials per
batch and transposes back ([o, s] -> [s, o]).

Device algorithm (per core, S=2048, D=1024, HD=128, 4 q heads, 1 kv):
  - QKV projections from xT [d, s] tiles (f32r matmuls); q in [s, (h hd)]
    layout for RMS-norm/RoPE (free-dim reductions), then PE-transposed to
    qT [hd, h, s]; k likewise to kT [hd, s]; k's rstd is folded into the
    exp() per-partition scale; q's rstd*gain*HD^-0.5 folded before transpose.
  - scoresT [sk, sq] = kT_tile.T @ qT slice per (j-block, head); exp on ACT
    with bf16 output; causal via restricting to valid sq range plus one
    triangular multiplicative mask on the diagonal 128-block.
  - attn@v with exp SUBTILES [sk, 128] as lhsT against v_aug [sk, 129] bf16
    whose last column is ones: y [sq, hd] accumulates in PSUM and column 128
    accumulates the softmax DENOMINATOR for free.
  - y -> yT transpose on PE via matmul with diag(1/den) bf16 (the division
    folds into the transpose); all PSUM evacuations run on Pool (gpsimd),
    which pays no PSUM access penalty.
  - Output projection per (oc, j) with wo bf16 stationary -> outT [o, s],
    interleaved after each attention block to fill PE gaps.
"""

import sys

for _p in ("/opt/trn_rl_repo",):
    if _p not in sys.path:
        sys.path.insert(0, _p)

import numpy as np

import concourse.bass as bass
import concourse.bacc as bacc
import concourse.mybir as mybir
import concourse.tile as tile
from concourse import bass_utils
from concourse.masks import make_identity

F32 = mybir.dt.float32
F32R = mybir.dt.float32r
BF16 = mybir.dt.bfloat16
AFT = mybir.ActivationFunctionType

B, S, D = 4, 2048, 1024
H, KVH, HD = 8, 2, 128
NH = H // KVH          # q heads per core = 4
RD, RH = 64, 32        # rope dims / half
NB, BLK = 4, 512       # s blocks
NT, TS = 16, 128       # s tiles
NDC = D // 128         # 8 d-chunks
VW = HD + 1            # v width incl ones column = 129
EPS = float(np.finfo(np.float32).eps)


def _build_nc(reps=1):
    nc = bacc.Bacc("TRN2", target_bir_lowering=False, debug=False,
                   enable_asserts=False)

    xT = nc.dram_tensor("xT", (D, S), F32R, kind="ExternalInput").ap()
    wq = nc.dram_tensor("wq", (128, NDC, NH * HD), F32R, kind="ExternalInput").ap()
    wkv = nc.dram_tensor("wkv", (128, NDC, 2 * HD), F32R,
                         kind="ExternalInput").ap()
    wo = nc.dram_tensor("wo", (128, NH, D), F32, kind="ExternalInput").ap()
    cosq = nc.dram_tensor("cosq", (128, NT, RH), F32, kind="ExternalInput").ap()
    sinq = nc.dram_tensor("sinq", (128, NT, RH), F32, kind="ExternalInput").ap()
    nsinq = nc.dram_tensor("nsinq", (128, NT, RH), F32, kind="ExternalInput").ap()
    qsc = nc.dram_tensor("qsc", (1, NH), F32, kind="ExternalInput").ap()
    if reps == 1:
        outT = nc.dram_tensor("outT", (D, S), F32, kind="ExternalOutput").ap()
        outs = [outT]
    else:
        outT = nc.dram_tensor("outT", (reps, D, S), F32,
                              kind="ExternalOutput").ap()
        outs = [outT[r] for r in range(reps)]

    with tile.TileContext(nc) as tc, \
         nc.allow_low_precision(reason="float32r/bf16 matmul operands"):
        for _rep in range(reps):
            _kern(nc, tc, xT, wq, wkv, wo, cosq, sinq, nsinq, qsc,
                  outs[_rep])
    nc.compile()
    return nc


def _kern(nc, tc, xT, wq, wkv, wo, cosq, sinq, nsinq, qsc, outT):
    mm = nc.tensor.matmul

    persist_cm = tc.tile_pool(name="persist", bufs=1)
    persist = persist_cm.__enter__()
    # ---- persistent tiles -------------------------------------------------
    wq_sb = persist.tile([128, NDC, NH * HD], F32R, tag="wq_sb", name="wq_sb")
    wkv_sb = persist.tile([128, NDC, 2 * HD], F32R, tag="wkv_sb",
                          name="wkv_sb")
    wo_sb = persist.tile([128, NH, D], BF16, tag="wo_sb", name="wo_sb")
    cosq_sb = persist.tile([128, NT, RH], F32, tag="cosq_sb", name="cosq_sb")
    nc.gpsimd.dma_start(out=cosq_sb, in_=cosq)
    sinq_sb = persist.tile([128, NT, RH], F32, tag="sinq_sb", name="sinq_sb")
    nc.gpsimd.dma_start(out=sinq_sb, in_=sinq)
    nsinq_sb = persist.tile([128, NT, RH], F32, tag="nsinq_sb", name="nsinq_sb")
    nc.gpsimd.dma_start(out=nsinq_sb, in_=nsinq)
    qsc_sb = persist.tile([128, NH], F32, tag="qsc_sb", name="qsc_sb")
    nc.gpsimd.dma_start(out=qsc_sb, in_=qsc.to_broadcast((128, NH)))

    eps_col = persist.tile([128, 1], F32, tag="eps_col", name="eps_col")
    nc.vector.memset(eps_col, EPS)
    ident_st = persist.tile([128, 128], F32, tag="ident_st", name="ident_st")
    make_identity(nc, ident_st)
    ident = persist.tile([128, 128], F32R, tag="ident", name="ident")
    nc.vector.tensor_copy(out=ident, in_=ident_st)
    # tri[r, c] = 1.0 if r <= c else 0.0  (causal keep-mask on the diagonal
    # of scoresT [sk, sq])
    tri = persist.tile([128, 128], BF16, tag="tri", name="tri")
    nc.gpsimd.memset(tri, 1.0)
    nc.gpsimd.affine_select(
        out=tri, in_=tri, compare_op=mybir.AluOpType.is_ge, fill=0.0,
        base=0, pattern=[[1, 128]], channel_multiplier=-1)
    # [zeros | tri]: keep-mask for the last diagonal tile padded to 256 wide
    # (avoids the f32r <256-row matmul penalty on the 128-wide slice)
    trz = persist.tile([128, 256], BF16, tag="trz", name="trz")
    nc.gpsimd.memset(trz[:, 0:128], 0.0)
    nc.gpsimd.tensor_copy(out=trz[:, 128:256], in_=tri)

    qT_sb = persist.tile([128, NH, S], F32R, tag="qT_sb", name="qT_sb")   # [hd, h, s]
    kT_sb = persist.tile([128, S], F32R, tag="kT_sb", name="kT_sb")       # [hd, s]
    v_sb = persist.tile([128, NT, VW], BF16, tag="v_sb", name="v_sb")     # [sk, t, hd+1]
    nc.gpsimd.memset(v_sb[:, :, HD:VW], 1.0)
    rstdk_sb = persist.tile([128, NT], F32, tag="rstdk_sb", name="rstdk_sb")
    yT_sb = persist.tile([128, NH, S], BF16, tag="yT_sb", name="yT_sb")   # [hd, h, s]

    # ---- phase 1: projections + norm + rope + transposes ------------------
    # PSUM budget (8 banks): qp x4 + kvp x2 + tp_ps x2.
    p1ps_cm = tc.tile_pool(name="p1_psum", bufs=1, space="PSUM")
    p1ps = p1ps_cm.__enter__()
    p1sb_cm = tc.tile_pool(name="p1_sbuf", bufs=1)
    p1sb = p1sb_cm.__enter__()

    # wq/wkv DMA directly into their f32r SBUF tiles on the SP queue,
    # interleaved per d-chunk pair (PE consumes them pairwise in-order).
    nc.sync.dma_start(out=wq_sb[:, 0:1, :], in_=wq[:, 0:1, :])
    nc.sync.dma_start(out=wkv_sb[:, 0:2, :], in_=wkv[:, 0:2, :])
    for p in range(NDC // 2):
        c0 = max(1, 2 * p)
        nc.sync.dma_start(out=wq_sb[:, c0:2 * p + 2, :],
                          in_=wq[:, c0:2 * p + 2, :])
        if p >= 1:
            nc.sync.dma_start(out=wkv_sb[:, 2 * p:2 * p + 2, :],
                              in_=wkv[:, 2 * p:2 * p + 2, :])

    def proc_q(i, qp, evac_eng=None):
        # qp: [128, 512] f32 PSUM = q for s-tile i, 4 heads x 128
        qpv = qp.rearrange("p (h f) -> p h f", h=NH)
        sqscr = p1sb.tile([128, BLK], F32, tag="sqscr", bufs=2,
                          name=f"sqscr_{i}")
        nc.scalar.activation(out=sqscr, in_=qp, func=AFT.Square)
        sumsq = p1sb.tile([128, NH], F32, tag="sumsq", bufs=3,
                          name=f"sumsq_{i}")
        nc.vector.tensor_reduce(
            out=sumsq, in_=sqscr.rearrange("p (h f) -> p h f", h=NH),
            axis=mybir.AxisListType.X, op=mybir.AluOpType.add)
        qsrt = p1sb.tile([128, NH], F32, tag="qsrt", bufs=3, name=f"qsrt_{i}")
        nc.scalar.activation(out=qsrt, in_=sumsq, func=AFT.Sqrt,
                             bias=eps_col, scale=1.0 / HD)
        rstd = p1sb.tile([128, NH], F32, tag="rstd", bufs=3, name=f"rstd_{i}")
        nc.vector.reciprocal(rstd, qsrt)
        rsc = p1sb.tile([128, NH], F32, tag="rsc", bufs=3, name=f"rsc_{i}")
        nc.vector.tensor_mul(rsc, rstd, qsc_sb)
        rsc_b = rsc[:, :, None].broadcast_to([128, NH, RD])

        cos_b = cosq_sb[:, i, None, :].broadcast_to([128, NH, RH])
        sin_b = sinq_sb[:, i, None, :].broadcast_to([128, NH, RH])
        nsin_b = nsinq_sb[:, i, None, :].broadcast_to([128, NH, RH])
        tcq = p1sb.tile([128, NH, RD], F32, tag="tcq", bufs=3,
                        name=f"tcq_{i}")
        tsq = p1sb.tile([128, NH, RD], F32, tag="tsq", bufs=3,
                        name=f"tsq_{i}")
        nc.vector.tensor_mul(tcq[:, :, 0:RH], qpv[:, :, 0:RH], cos_b)
        nc.vector.tensor_mul(tcq[:, :, RH:RD], qpv[:, :, RH:RD], cos_b)
        nc.vector.tensor_mul(tsq[:, :, 0:RH], qpv[:, :, RH:RD], sin_b)
        nc.vector.tensor_mul(tsq[:, :, RH:RD], qpv[:, :, 0:RH], nsin_b)

        qstage = p1sb.tile([128, BLK], F32R, tag="qstage", bufs=5,
                           name=f"qstage_{i}")
        qsv = qstage.rearrange("p (h f) -> p h f", h=NH)
        nc.gpsimd.tensor_add(qsv[:, :, 0:RD], tcq, tsq)
        nc.gpsimd.tensor_mul(qsv[:, :, 0:RD], qsv[:, :, 0:RD], rsc_b)
        nc.vector.tensor_mul(qsv[:, :, RD:HD], qpv[:, :, RD:HD],
                             rsc[:, :, None].broadcast_to(
                                 [128, NH, HD - RD]))

        def transpose_unit():
            tp = p1ps.tile([128, NH, 128], F32R, tag="tp_ps", bufs=2,
                           name=f"qtp_{i}")
            for h in range(NH):
                nc.tensor.transpose(tp[:, h, :],
                                    qstage[:, h * 128:(h + 1) * 128], ident)
            if evac_eng == "dve":
                nc.vector.tensor_copy(
                    out=qT_sb[:, :, i * 128:(i + 1) * 128], in_=tp)
            else:
                nc.scalar.activation(
                    out=qT_sb[:, :, i * 128:(i + 1) * 128], in_=tp,
                    func=AFT.Copy)
        return transpose_unit

    def proc_k(t, kp):
        # kp: [128, 128] f32 PSUM view = k for s-tile t
        ksq = p1sb.tile([128, HD], F32, tag="ksq", bufs=3, name=f"ksq_{t}")
        ksum = p1sb.tile([128, 1], F32, tag="ksum", bufs=3, name=f"ksum_{t}")
        nc.scalar.activation(out=ksq, in_=kp, func=AFT.Square,
                             accum_out=ksum)
        ksrt = p1sb.tile([128, 1], F32, tag="ksrt", bufs=3, name=f"ksrt_{t}")
        nc.scalar.activation(out=ksrt, in_=ksum, func=AFT.Sqrt,
                             bias=eps_col, scale=1.0 / HD)
        nc.vector.reciprocal(rstdk_sb[:, t:t + 1], ksrt)
        tckk = p1sb.tile([128, RD], F32, tag="tckk", bufs=3, name=f"tckk_{t}")
        tskk = p1sb.tile([128, RD], F32, tag="tskk", bufs=3, name=f"tskk_{t}")
        nc.vector.tensor_mul(tckk[:, 0:RH], kp[:, 0:RH], cosq_sb[:, t, :])
        nc.vector.tensor_mul(tckk[:, RH:RD], kp[:, RH:RD], cosq_sb[:, t, :])
        nc.vector.tensor_mul(tskk[:, 0:RH], kp[:, RH:RD], sinq_sb[:, t, :])
        nc.vector.tensor_mul(tskk[:, RH:RD], kp[:, 0:RH], nsinq_sb[:, t, :])
        kstage = p1sb.tile([128, HD], F32R, tag="kstage", bufs=5,
                           name=f"kstage_{t}")
        nc.gpsimd.tensor_add(kstage[:, 0:RD], tckk, tskk)
        nc.vector.tensor_copy(out=kstage[:, RD:HD], in_=kp[:, RD:HD])
        return kstage

    # x loads: even d-chunks on the ACT queue, odd on SP, batched multi-chunk
    # DMAs from a strided DRAM view (b=0 split finer for startup granularity).
    xr = xT.rearrange("(c two p) s -> p two c s", two=2, p=128)
    tr_pending = []   # transpose closures lagged one half-block

    def drain_tr():
        for fn_ in tr_pending:
            fn_()
        del tr_pending[:]

    for b in range(NB):
        sl = slice(b * BLK, (b + 1) * BLK)
        xsl_of = {}
        if b == 0:
            # finer startup granularity: two half-DMAs per staging tile,
            # everything on the ACT queue (SP is busy with weights)
            xst2 = []
            for two in range(2):
                xst = p1sb.tile([128, 4, BLK], F32R, tag=f"xt_st{two}",
                                bufs=2, name=f"xtst_{b}_{two}")
                xst2.append(xst)
                for c in range(4):
                    xsl_of[c * 2 + two] = xst[:, c, :]
            for c0 in range(0, 4, 2):
                for two in range(2):
                    nc.scalar.dma_start(out=xst2[two][:, c0:c0 + 2, :],
                                        in_=xr[:, two, c0:c0 + 2, sl])
        else:
            for two, eng in ((0, nc.scalar), (1, nc.sync)):
                xst = p1sb.tile([128, 4, BLK], F32R, tag=f"xt_st{two}",
                                bufs=2, name=f"xtst_{b}_{two}")
                eng.dma_start(out=xst, in_=xr[:, two, :, sl])
                for c in range(4):
                    xsl_of[c * 2 + two] = xst[:, c, :]

        for half in range(2):
            qps2 = [p1ps.tile([128, BLK], F32, tag="q_ps", bufs=3,
                              name=f"qps_{b}_{half}_{x}")
                    for x in range(2)]
            kvps2 = [p1ps.tile([128, 2 * HD], F32, tag="kv_ps", bufs=2,
                               name=f"kvps_{b}_{half}_{x}")
                     for x in range(2)]
            for di in range(NDC):
                st, sp = di == 0, di == NDC - 1
                for x in range(2):
                    ii = half * 2 + x
                    xsl = xsl_of[di][:, ii * 128:(ii + 1) * 128]
                    mm(qps2[x], lhsT=xsl, rhs=wq_sb[:, di, :],
                       start=st, stop=sp)
                    mm(kvps2[x], lhsT=xsl, rhs=wkv_sb[:, di, :],
                       start=st, stop=sp)
            drain_tr()
            kst2 = []
            qtr2 = []
            for x in range(2):
                t = b * 4 + half * 2 + x
                kst2.append(proc_k(t, kvps2[x][:, 0:HD]))
            for x in range(2):
                t = b * 4 + half * 2 + x
                nc.vector.tensor_copy(out=v_sb[:, t, 0:HD],
                                      in_=kvps2[x][:, HD:2 * HD])
                qtr2.append(proc_q(t, qps2[x]))

            def k_unit(b=b, half=half, kst2=kst2):
                evac_dve = False
                ktp = p1ps.tile([128, 2, 128], F32R, tag="ktp_ps", bufs=1,
                                name=f"ktp_{b}_{half}")
                for x in range(2):
                    nc.tensor.transpose(ktp[:, x, :], kst2[x], ident)
                off = b * BLK + half * 256
                if evac_dve:
                    nc.vector.tensor_copy(
                        out=kT_sb[:, off:off + 256],
                        in_=ktp.rearrange("p h f -> p (h f)"))
                else:
                    nc.scalar.activation(
                        out=kT_sb[:, off:off + 256],
                        in_=ktp.rearrange("p h f -> p (h f)"), func=AFT.Copy)
            tr_pending.append(k_unit)
            tr_pending.extend(qtr2)
        if b == 1:
            for ch in range(NH):
                wst = p1sb.tile([128, D], F32, tag="wst_wo", bufs=1,
                                name=f"wst_wo_{ch}")
                nc.sync.dma_start(out=wst, in_=wo[:, ch, :])
                nc.gpsimd.tensor_copy(out=wo_sb[:, ch, :], in_=wst)
    drain_tr()

    p1sb_cm.__exit__(None, None, None)
    p1ps_cm.__exit__(None, None, None)

    # ---- phase 2: attention + interleaved output projection ---------------
    # PSUM budget: sc_ps [128,1024] x2 (4 banks) + ypack [128,3,129] x3
    # (3 banks) + big_ps [128,512] x1 (1 bank, shared by yT transposes and
    # the out-projection).
    p2ps_cm = tc.tile_pool(name="p2_psum", bufs=1, space="PSUM")
    p2ps = p2ps_cm.__enter__()
    p2sb_cm = tc.tile_pool(name="p2_sbuf", bufs=1)
    p2sb = p2sb_cm.__enter__()

    # Deferred PE-heavy units (pass-2 chains, yT transposes, out-proj steps)
    # are drained into the NEXT pass-1's emission so PE's in-order stream has
    # filler work during the ACT-bound exp stretches.
    pending = []

    def drain(k):
        for _ in range(k):
            if pending:
                pending.pop(0)()

    def outproj_unit(j, oc):
        def emit():
            jsl = slice(j * BLK, (j + 1) * BLK)
            osl = slice(oc * 128, (oc + 1) * 128)
            ops_ = p2ps.tile([128, BLK], F32, tag="big_ps", bufs=2,
                             name=f"ops_{j}_{oc}")
            for c in range(NH):
                mm(ops_, lhsT=wo_sb[:, c, osl], rhs=yT_sb[:, c, jsl],
                   start=(c == 0), stop=(c == NH - 1))
            oc_sb = p2sb.tile([128, BLK], F32, tag="oc_sb", bufs=4,
                              name=f"ocsb_{j}_{oc}")
            nc.vector.tensor_copy(out=oc_sb, in_=ops_)
            nc.sync.dma_start(out=outT[osl, jsl], in_=oc_sb)
        return emit

    def chain_unit(j, hp, hl, u, expts, den, ysb):
        def emit():
            idx = hl * 4 + u
            yp = p2ps.tile([128, VW], F32, tag="y_ps", bufs=2,
                           name=f"yp_{j}_{hp}_{idx}")
            for t in range(4 * j + u + 1):
                mm(yp, lhsT=expts[t][:, hl, u * 128:(u + 1) * 128],
                   rhs=v_sb[:, t, :],
                   start=(t == 0), stop=(t == 4 * j + u))
            nc.vector.tensor_copy(out=den[:, idx:idx + 1], in_=yp[:, HD:VW])
            nc.vector.tensor_copy(out=ysb[:, idx, :], in_=yp[:, 0:HD])
        return emit

    def finish_unit(j, hp, den, ysb):
        def emit():
            rec = p2sb.tile([128, 8], F32, tag="rec", bufs=3,
                            name=f"rec_{j}_{hp}")
            nc.vector.reciprocal(rec, den)
            for hl in range(2):
                ytp = p2ps.tile([128, BLK], F32, tag="big_ps", bufs=2,
                                name=f"ytp_{j}_{hp}_{hl}")
                for u in range(4):
                    idx = hl * 4 + u
                    dg = p2sb.tile([128, 128], BF16, tag="dg", bufs=8,
                                   name=f"dg_{j}_{hp}_{hl}_{u}")
                    nc.gpsimd.affine_select(
                        out=dg,
                        in_=rec[:, idx:idx + 1].broadcast_to([128, 128]),
                        compare_op=mybir.AluOpType.is_equal, fill=0.0,
                        base=0, pattern=[[1, 128]], channel_multiplier=-1)
                    mm(ytp[:, u * 128:(u + 1) * 128],
                       lhsT=ysb[:, idx, :], rhs=dg,
                       start=True, stop=True)
                h = hp * 2 + hl
                nc.vector.tensor_copy(
                    out=yT_sb[:, h, j * BLK:(j + 1) * BLK], in_=ytp)
        return emit

    for j in range(NB):
        n_t = 4 * j + 4
        for hp in range(2):
            # ---- pass 1: scores + exp for all key tiles (kept in SBUF) ----
            expts = []
            for t in range(n_t):
                diag = t >= 4 * j
                m = (t - 4 * j) * 128 if diag else 0
                me = 256 if m == 384 else m
                tsl = slice(t * 128, (t + 1) * 128)
                sc2 = p2ps.tile([128, 2, BLK], F32, tag="sc_ps",
                                bufs=2, name=f"sc2_{j}_{hp}_{t}")
                expt2 = p2sb.tile([128, 2, BLK], BF16, tag=f"expt{hp}",
                                  bufs=NT, name=f"expt_{j}_{hp}_{t}")
                for hh in range(2):
                    h = hp * 2 + hh
                    mm(sc2[:, hh, me:BLK],
                       lhsT=kT_sb[:, tsl],
                       rhs=qT_sb[:, h, j * BLK + me:(j + 1) * BLK],
                       start=True, stop=True)
                nc.scalar.activation(out=expt2[:, :, me:BLK],
                                     in_=sc2[:, :, me:BLK], func=AFT.Exp,
                                     scale=rstdk_sb[:, t:t + 1])
                if diag and m == 384:
                    nc.vector.tensor_mul(
                        expt2[:, :, 256:BLK], expt2[:, :, 256:BLK],
                        trz[:, None, :].broadcast_to([128, 2, 256]))
                elif diag:
                    nc.vector.tensor_mul(
                        expt2[:, :, m:m + 128], expt2[:, :, m:m + 128],
                        tri[:, None, :].broadcast_to([128, 2, 128]))
                expts.append(expt2)
                slots_left = n_t - t
                if len(pending) >= slots_left:
                    drain(min(3, -(-len(pending) // slots_left)))

            # ---- pass 2 (deferred): y[sq, hd] + free denominator column ---
            den = p2sb.tile([128, 8], F32, tag="den", bufs=3,
                            name=f"den_{j}_{hp}")
            ysb = p2sb.tile([128, 8, HD], BF16, tag="ysb", bufs=2,
                            name=f"ysb_{j}_{hp}")
            for hl in range(2):
                for u in range(4):
                    pending.append(chain_unit(j, hp, hl, u, expts, den, ysb))
            pending.append(finish_unit(j, hp, den, ysb))
        for oc in range(8):
            pending.append(outproj_unit(j, oc))
    drain(len(pending))

    p2sb_cm.__exit__(None, None, None)
    p2ps_cm.__exit__(None, None, None)

    persist_cm.__exit__(None, None, None)


_NC_CACHE = {}


def _get_nc():
    if "nc" not in _NC_CACHE:
        _NC_CACHE["nc"] = _build_nc()
    return _NC_CACHE["nc"]


def _host_tables():
    pos = np.arange(S, dtype=np.float32)
    inv = (1.0 / (10000.0 ** (np.arange(0, RD, 2, dtype=np.float32) / RD)))
    fr = np.outer(pos, inv).astype(np.float32)          # [S, 32]
    cos, sin = np.cos(fr), np.sin(fr)
    tile128 = lambda a: np.ascontiguousarray(
        a.reshape(NT, 128, RH).transpose(1, 0, 2))      # [128, NT, 32]
    return tile128(cos), tile128(sin), tile128(-sin)


def _make_in_maps(inputs):
    x = np.asarray(inputs["x"], dtype=np.float32)
    w_q = np.asarray(inputs["w_q"], dtype=np.float32)
    w_k = np.asarray(inputs["w_k"], dtype=np.float32)
    w_v = np.asarray(inputs["w_v"], dtype=np.float32)
    w_o = np.asarray(inputs["w_o"], dtype=np.float32)
    q_gain = np.asarray(inputs["q_gain"], dtype=np.float32)

    cosq, sinq, nsinq = _host_tables()

    def wtile(wT, chunks, width):
        # [chunks*128, width] -> [128, chunks, width]
        return np.ascontiguousarray(
            wT.reshape(chunks, 128, width).transpose(1, 0, 2))

    in_maps = []
    for core in range(8):
        b, g = divmod(core, 2)
        cols = slice(g * NH * HD, (g + 1) * NH * HD)
        xTc = np.ascontiguousarray(x[b].T)                       # [D, S]
        wq_t = wtile(np.ascontiguousarray(w_q[cols, :].T), NDC, NH * HD)
        wkv_t = wtile(np.ascontiguousarray(np.concatenate(
            [w_k[g * HD:(g + 1) * HD, :].T, w_v[g * HD:(g + 1) * HD, :].T],
            axis=1)), NDC, 2 * HD)
        wo_t = wtile(np.ascontiguousarray(w_o[:, cols].T), NH, D)
        qsc_h = (q_gain[g * NH:(g + 1) * NH] *
                 np.float32(HD ** -0.5)).astype(np.float32).reshape(1, NH)
        in_maps.append(dict(
            xT=xTc, wq=wq_t, wkv=wkv_t, wo=wo_t,
            cosq=cosq, sinq=sinq, nsinq=nsinq, qsc=qsc_h))
    return in_maps


def kernel(x, w_q, w_k, w_v, w_o, q_gain):
    nc = _get_nc()
    in_maps = _make_in_maps(dict(x=x, w_q=w_q, w_k=w_k, w_v=w_v, w_o=w_o,
                                 q_gain=q_gain))
    res = bass_utils.run_bass_kernel_spmd(nc, in_maps,
                                          core_ids=list(range(8)))
    out = np.empty((B, S, D), dtype=np.float32)
    for b in range(B):
        p0 = res.results[2 * b]["outT"]
        p1 = res.results[2 * b + 1]["outT"]
        out[b] = (p0 + p1).T
    return out
